# revision 33
# baseline (speedup 1.0000x reference)
"""GATv2 x2 + edge decoder (gnn_message_passing) on 8 TRN2 NeuronCores.

v4 design (bf16, gather-accumulate, dst-sharded edges):
- Per layer, dense phase computes TWO node tables in DRAM (bf16):
  xl_tab = x @ Wl.T (no bias; alpha sums to 1 so bl folds into the output
  bias) and xr_tab = x @ Wr.T (+ bl+br folded in, covering the m-bias).
- Edge phase (dst-sorted edges, 128-edge subtiles, core k owns dst nodes
  [k*N/8,(k+1)*N/8)): per subtile one indirect DMA gathers xl[src] rows and
  a second indirect DMA with compute_op=add accumulates xr[dst] in-flight,
  producing m = xl[src]+xr[dst] directly in SBUF with zero compute ops.
- Scores: q = Lrelu(m) on ACT, tq = q*att and the per-head reduce on DVE,
  batched over G subtiles per instruction. exp on ACT.
- Scatter (segment softmax sum): one matmul per subtile with the 0/1 matrix
  sel[e,n] = (dstloc_e == n) as the stationary operand accumulates both
  out += sel.T @ (xl[src]*ex) and den += sel.T @ ex in PSUM; the divide by
  den happens once per 128-node dst tile.
- Host-side node re-permutation equalizes per-dst-tile edge counts
  (S_sub = 18 instead of 19).
- z is AllGathered between layers (bf16); layer-2/PQ lhsT tiles are loaded
  with DMA-transpose.
- Decoder: P/Q node tables (256B bf16 rows); per 128-edge subtile one
  gather P[src] + one CCE-add gather Q[dst], then lrelu/dot/reduce on DVE
  batched over 4 subtiles.
"""

import sys

sys.path.insert(0, "/opt/trn_rl_repo")

import numpy as np

import bass_rust
import concourse.bass as bass
import concourse.mybir as mybir
import concourse.tile as tile

P = 128
NS_ATT = 0.2
NS_ACT = 0.01
dt = mybir.dt
Alu = mybir.AluOpType
Act = mybir.ActivationFunctionType


# ---------------------------------------------------------------------------
# workaround: this walrus build rejects sem waits attached to InstDrain
# ("Too many sync wait commands"); hoist every drain wait onto NoOps.
def _fix_waits(nc, max_other=1):
    for bb in nc.main_func.blocks:
        newlist = []
        for ins in bb.instructions:
            si = ins.sync_info
            if si is not None and si.on_wait:
                waits = list(si.on_wait)
                no_wait = isinstance(ins, mybir.InstDrain) or hasattr(ins, "isa_opcode")
                limit = 0 if no_wait else max_other
                if len(waits) > limit:
                    nkeep = limit
                    extra = waits[: len(waits) - nkeep] if nkeep else waits
                    keep = waits[len(waits) - nkeep:] if nkeep else []
                    k = 0
                    while extra:
                        chunk, extra = extra[:1], extra[1:]
                        nop = mybir.InstNoOp(
                            name=f"{ins.name}_ws{k}", engine=ins.engine, ins=[], outs=[]
                        )
                        nop.sync_info = bass_rust.SyncInfo(on_wait=chunk, on_update=[])
                        newlist.append(nop)
                        k += 1
                    ins.sync_info = bass_rust.SyncInfo(
                        on_wait=keep, on_update=list(si.on_update or [])
                    )
            newlist.append(ins)
        bb.instructions = newlist


def _batches(S, G):
    """Split S subtiles into batches of size <= G, e.g. 19,6 -> [7,6,6]."""
    nb = (S + G - 1) // G
    base, rem = divmod(S, nb)
    return [base + (1 if i < rem else 0) for i in range(nb)]


def build_program(cfg, fix=True, dbg=False):
    N, C, H = cfg["N"], cfg["C"], cfg["H"]
    NC_ = cfg["n_cores"]
    HC = H * C
    NPC = N // NC_
    T = NPC // P
    TN = N // P
    S = cfg["S_sub"]
    G = cfg["G"]
    GD = cfg["GD"]
    DSUB = cfg["DSUB"]
    BD = 4                      # dense-phase tile batch
    has_mbias1 = cfg["has_mbias1"]
    has_mbias2 = cfg["has_mbias2"]
    has_bd1 = cfg["has_bd1"]
    bat = _batches(S, G)
    Gmax = bat[0]

    nc = bass.Bass()
    bf = dt.bfloat16
    f32 = dt.float32

    def inp(name, shape, d=bf):
        return nc.declare_dram_parameter(name, list(shape), d, isOutput=False)

    xT = inp("xT", [C, N])
    wlT1 = inp("wlT1", [C, HC])
    wrT1 = inp("wrT1", [C, HC])
    wlT2 = inp("wlT2", [C, HC])
    wrT2 = inp("wrT2", [C, HC])
    att1r = inp("att1r", [P, Gmax * HC])
    att2r = inp("att2r", [P, Gmax * HC])
    iotar = inp("iotar", [P, Gmax * P])
    dstloc = inp("dstloc", [P, T * S])
    srcI = inp("srcI", [P, T * S], dt.int32)
    dstI = inp("dstI", [P, T * S], dt.int32)
    bias1r = inp("bias1r", [P, C], f32)
    bias2r = inp("bias2r", [P, C], f32)
    blr1 = inp("blr1", [1, HC], f32)     # bl1+br1 (m-bias layer 1)
    blr2 = inp("blr2", [1, HC], f32)
    onesr = inp("onesr", [1, P], f32)
    abT = inp("abT", [C, 2 * C])
    bd1r = inp("bd1r", [1, 2 * C], f32)
    wd2r = inp("wd2r", [P, GD * C])
    decS = inp("decS", [P, DSUB], dt.int32)
    decD = inp("decD", [P, DSUB], dt.int32)
    selfI = inp("selfI", [P, T], dt.int32)
    iotac = inp("iotac", [P, 1], f32)

    out_dec = nc.declare_dram_parameter("out_dec", [DSUB, P], f32, isOutput=True)
    if dbg:
        d_xl1 = nc.declare_dram_parameter("d_xl1", [2 * P, HC], f32, isOutput=True)
        d_xr1 = nc.declare_dram_parameter("d_xr1", [2 * P, HC], f32, isOutput=True)
        d_gl = nc.declare_dram_parameter("d_gl", [P, 6 * HC], f32, isOutput=True)
        d_sel = nc.declare_dram_parameter("d_sel", [P, 6 * P], f32, isOutput=True)
        d_q = nc.declare_dram_parameter("d_q", [P, 6 * HC], f32, isOutput=True)
        d_s8 = nc.declare_dram_parameter("d_s8", [P, 6 * H], f32, isOutput=True)
        d_ex = nc.declare_dram_parameter("d_ex", [P, 6 * H], f32, isOutput=True)
        d_msg = nc.declare_dram_parameter("d_msg", [P, 6 * HC], f32, isOutput=True)
        d_den = nc.declare_dram_parameter("d_den", [P, H], f32, isOutput=True)
        d_z = nc.declare_dram_parameter("d_z", [NPC, C], f32, isOutput=True)
        d_zall = nc.declare_dram_parameter("d_zall", [2 * P, C], f32, isOutput=True)
        d_p = nc.declare_dram_parameter("d_p", [2 * P, C], f32, isOutput=True)
        d_res = nc.declare_dram_parameter("d_res", [P, DSUB], f32, isOutput=True)
        d_o = nc.declare_dram_parameter("d_o", [P, HC], f32, isOutput=True)
        d_zsum = nc.declare_dram_parameter("d_zsum", [P, C], f32, isOutput=True)
        d_rden = nc.declare_dram_parameter("d_rden", [P, H], f32, isOutput=True)
        d_outps = nc.declare_dram_parameter("d_outps", [P, HC], f32, isOutput=True)
        d_outp1 = nc.declare_dram_parameter("d_outp1", [P, HC], f32, isOutput=True)

    with tile.TileContext(nc) as tc:
        with (
            tc.tile_pool(name="const", bufs=1) as cp,
            tc.tile_pool(name="work", bufs=2) as wp,
            tc.tile_pool(name="msgp", bufs=2) as mp,
            tc.tile_pool(name="gpool", bufs=2) as gp,
            tc.tile_pool(name="psum", bufs=3, space="PSUM") as pp,
            tc.tile_pool(name="psum1", bufs=2, space="PSUM") as pp1,
            tc.tile_pool(name="dram", bufs=1, space="DRAM") as dp,
        ):
            def cload(ap, shape, d=bf):
                t_ = cp.tile(list(shape), d, tag=f"c_{ap.name}")
                nc.sync.dma_start(out=t_[:], in_=ap[:, :])
                return t_

            wl1_sb = cload(wlT1, [C, HC])
            wr1_sb = cload(wrT1, [C, HC])
            wl2_sb = cload(wlT2, [C, HC])
            wr2_sb = cload(wrT2, [C, HC])
            def load_att(ap):
                t_ = cp.tile([P, Gmax * HC], bf, tag="c_att")
                nc.sync.dma_start(out=t_[:], in_=ap[:, :])
                return t_

            att1_sb = load_att(att1r)
            iota_sb = cload(iotar, [P, Gmax * P])
            dstloc_sb = cload(dstloc, [P, T * S])
            srcI_sb = cload(srcI, [P, T * S], dt.int32)
            dstI_sb = cload(dstI, [P, T * S], dt.int32)
            bias1_sb = cload(bias1r, [P, C], f32)
            bias2_sb = cload(bias2r, [P, C], f32)
            blr1_sb = cload(blr1, [1, HC], f32)
            blr2_sb = cload(blr2, [1, HC], f32)
            ones_sb = cload(onesr, [1, P], f32)
            abT_sb = cload(abT, [C, 2 * C])
            bd1_sb = cload(bd1r, [1, 2 * C], f32)
            wd2_sb = cload(wd2r, [P, GD * C])
            decS_sb = cload(decS, [P, DSUB], dt.int32)
            selfI_sb = cload(selfI, [P, T], dt.int32)
            decD_sb = cload(decD, [P, DSUB], dt.int32)
            res_sb = cp.tile([P, DSUB], f32, tag="res_sb")

            xl1_tab = dp.tile([N, HC], bf)
            xr1_tab = dp.tile([N, HC], bf)
            xl2_tab = dp.tile([N, HC], bf)
            xr2_tab = dp.tile([N, HC], bf)
            z_loc = dp.tile([NPC, C], bf)
            z_all = dp.tile([NC_ * NPC, C], bf)
            z2_loc = dp.tile([NPC, C], bf)
            z2_all = dp.tile([NC_ * NPC, C], bf)
            p_tab = dp.tile([N, C], bf)
            q_tab = dp.tile([N, C], bf)

            # ---------------- dense phase: xl/xr tables --------------------
            def dense_phase(wlT_sb, wrT_sb, mbias_sb, has_mbias, xl_tab, xr_tab,
                            load_lhsT):
                for b0 in range(0, TN, BD):
                    lt = wp.tile([C, BD * P], bf, tag="lhsT")
                    load_lhsT(lt, b0)
                    xsl = wp.tile([P, BD * HC], bf, tag="xsl")
                    xsr = wp.tile([P, BD * HC], bf, tag="xsr")
                    for i in range(BD):
                        psl = pp.tile([P, HC], f32, tag="big")
                        for n0 in range(0, HC, 512):
                            nc.tensor.matmul(out=psl[:, n0:n0 + 512],
                                             lhsT=lt[:, i * P:(i + 1) * P],
                                             rhs=wlT_sb[:, n0:n0 + 512],
                                             start=True, stop=True)
                        psr = pp.tile([P, HC], f32, tag="big")
                        if has_mbias:
                            for n0 in range(0, HC, 512):
                                nc.tensor.matmul(out=psr[:, n0:n0 + 512],
                                                 lhsT=ones_sb[:, :],
                                                 rhs=mbias_sb[:, n0:n0 + 512],
                                                 start=True, stop=False)
                            for n0 in range(0, HC, 512):
                                nc.tensor.matmul(out=psr[:, n0:n0 + 512],
                                                 lhsT=lt[:, i * P:(i + 1) * P],
                                                 rhs=wrT_sb[:, n0:n0 + 512],
                                                 start=False, stop=True)
                        else:
                            for n0 in range(0, HC, 512):
                                nc.tensor.matmul(out=psr[:, n0:n0 + 512],
                                                 lhsT=lt[:, i * P:(i + 1) * P],
                                                 rhs=wrT_sb[:, n0:n0 + 512],
                                                 start=True, stop=True)
                        nc.scalar.activation(out=xsl[:, i * HC:(i + 1) * HC],
                                             in_=psl[:], func=Act.Copy)
                        nc.scalar.activation(out=xsr[:, i * HC:(i + 1) * HC],
                                             in_=psr[:], func=Act.Copy)
                    rows = slice(b0 * P, (b0 + BD) * P)
                    nc.sync.dma_start(
                        out=xl_tab[rows, :].rearrange("(b p) c -> p b c", p=P),
                        in_=xsl[:].rearrange("p (b c) -> p b c", b=BD))
                    nc.sync.dma_start(
                        out=xr_tab[rows, :].rearrange("(b p) c -> p b c", p=P),
                        in_=xsr[:].rearrange("p (b c) -> p b c", b=BD))

            # ---------------- edge phase ----------------------------------
            def edge_phase(xl_tab, xr_tab, att_sb, bias_sb, z_out, act):
                layer1 = z_out is z_loc
                inv_h = 1.0 / H
                for t in range(T):
                    tap = dbg and layer1 and t == 0
                    out_ps = pp.tile([P, HC], f32, tag="big")
                    den_ps = pp1.tile([P, 2 * C], f32, tag="small")
                    s = 0
                    for g in bat:
                        col0 = t * S + s
                        gl = gp.tile([P, Gmax, HC], bf, tag="gl")
                        for j in range(g):
                            nc.gpsimd.indirect_dma_start(
                                out=gl[:, j, :], out_offset=None, in_=xl_tab[:, :],
                                in_offset=bass.IndirectOffsetOnAxis(
                                    ap=srcI_sb[:, col0 + j:col0 + j + 1], axis=0))
                            nc.gpsimd.indirect_dma_start(
                                out=gl[:, j, :], out_offset=None, in_=xr_tab[:, :],
                                in_offset=bass.IndirectOffsetOnAxis(
                                    ap=dstI_sb[:, col0 + j:col0 + j + 1], axis=0),
                                compute_op=Alu.add)
                        sel = wp.tile([P, Gmax * P], bf, tag="sel")
                        nc.vector.tensor_tensor(
                            out=sel[:, :g * P].rearrange("p (g n) -> p g n", g=g),
                            in0=dstloc_sb[:, col0:col0 + g].to_broadcast([P, g, P]),
                            in1=iota_sb[:, :g * P].rearrange("p (g n) -> p g n", g=g),
                            op=Alu.is_equal)
                        q = wp.tile([P, Gmax * HC], bf, tag="q")
                        nc.scalar.activation(
                            out=q[:, :g * HC],
                            in_=gl[:, :, :].rearrange("p g c -> p (g c)")[:, :g * HC],
                            func=Act.Prelu, alpha=NS_ATT)
                        tq = wp.tile([P, Gmax * HC], bf, tag="tq")
                        nc.vector.tensor_tensor(out=tq[:, :g * HC], in0=q[:, :g * HC],
                                                in1=att_sb[:, :g * HC], op=Alu.mult)
                        s8 = wp.tile([P, Gmax * H], f32, tag="s8")
                        nc.vector.tensor_reduce(
                            out=s8[:, :g * H],
                            in_=tq[:, :g * HC].rearrange(
                                "p (gh c) -> p gh c", c=C),
                            axis=mybir.AxisListType.X, op=Alu.add)
                        ex = wp.tile([P, Gmax * H], bf, tag="ex")
                        nc.scalar.activation(out=ex[:, :g * H], in_=s8[:, :g * H],
                                             func=Act.Exp)
                        msg = mp.tile([P, Gmax * HC], bf, tag="msg")
                        nc.vector.tensor_tensor(
                            out=msg[:, :g * HC].rearrange("p (g h c) -> p g h c",
                                                          g=g, h=H),
                            in0=gl[:, 0:g, :].rearrange("p g (h c) -> p g h c", h=H),
                            in1=ex[:, :g * H].rearrange("p (g h) -> p g h", g=g)
                                .to_broadcast([P, g, H, C]),
                            op=Alu.mult)
                        for j in range(g):
                            first, last = (s + j == 0), (s + j == S - 1)
                            if tap and s + j == 1:
                                o1 = wp.tile([P, HC], f32, tag="dbgo")
                                nc.vector.tensor_copy(out=o1[:], in_=out_ps[:])
                                nc.sync.dma_start(out=d_outp1[:, :], in_=o1[:, :HC])
                            for n0 in range(0, HC, 512):
                                nc.tensor.matmul(
                                    out=out_ps[:, n0:n0 + 512],
                                    lhsT=sel[:, j * P:(j + 1) * P],
                                    rhs=msg[:, j * HC + n0:j * HC + n0 + 512],
                                    start=first, stop=last)
                            nc.tensor.matmul(
                                out=den_ps[:, :H],
                                lhsT=sel[:, j * P:(j + 1) * P],
                                rhs=ex[:, j * H:(j + 1) * H],
                                start=first, stop=last)
                        if tap and s == 0:
                            def dmp(dst, src_bf, width):
                                w2 = min(width, HC)
                                tmp = wp.tile([P, HC], f32, tag="dbgo")
                                nc.vector.tensor_copy(out=tmp[:, :w2], in_=src_bf[:, :w2])
                                nc.sync.dma_start(out=dst[:, 0:w2], in_=tmp[:, :w2])
                            dmp(d_gl, msg[:, 2 * HC:], 4 * HC)
                            dmp(d_sel, sel[:, :g * P], g * P)
                            dmp(d_q, msg[:, 4 * HC:], 2 * HC)
                            dmp(d_s8, s8[:, :g * H], g * H)
                            dmp(d_ex, ex[:, :g * H], g * H)
                            dmp(d_msg, msg[:, :g * HC], g * HC)
                        s += g
                    xrt = wp.tile([P, HC], bf, tag="xrt")
                    nc.gpsimd.indirect_dma_start(
                        out=xrt[:], out_offset=None, in_=xr_tab[:, :],
                        in_offset=bass.IndirectOffsetOnAxis(
                            ap=selfI_sb[:, t:t + 1], axis=0))
                    rden = wp.tile([P, H], f32, tag="rden")
                    if tap:
                        ot = wp.tile([P, HC], f32, tag="dbgo")
                        nc.vector.tensor_copy(out=ot[:], in_=out_ps[:])
                        nc.sync.dma_start(out=d_outps[:, :], in_=ot[:])
                        dent = wp.tile([P, H], f32, tag="dent")
                        nc.vector.tensor_copy(out=dent[:], in_=den_ps[:, :H])
                        nc.sync.dma_start(out=d_den[:, :], in_=dent[:])
                    nc.vector.reciprocal(out=rden[:], in_=den_ps[:, :H])
                    o_sb = wp.tile([P, HC], bf, tag="o")
                    nc.vector.tensor_tensor(
                        out=o_sb[:].rearrange("p (h c) -> p h c", h=H),
                        in0=out_ps[:].rearrange("p (h c) -> p h c", h=H),
                        in1=rden[:].to_broadcast([P, H, C]), op=Alu.mult)
                    if tap:
                        nc.sync.dma_start(out=d_rden[:, :], in_=rden[:])
                        ot2 = wp.tile([P, HC], f32, tag="dbgo")
                        nc.vector.tensor_copy(out=ot2[:], in_=o_sb[:])
                        nc.sync.dma_start(out=d_o[:, :], in_=ot2[:])
                    o2 = wp.tile([P, HC], bf, tag="o2")
                    nc.vector.tensor_tensor(out=o2[:], in0=o_sb[:], in1=xrt[:],
                                            op=Alu.subtract)
                    zsum = wp.tile([P, C], f32, tag="zsum")
                    nc.vector.tensor_reduce(
                        out=zsum[:], in_=o2[:].rearrange("p (h c) -> p c h", h=H),
                        axis=mybir.AxisListType.X, op=Alu.add)
                    if tap:
                        nc.sync.dma_start(out=d_zsum[:, :], in_=zsum[:])
                    zt = wp.tile([P, C], f32 if act else bf, tag="zt")
                    nc.vector.scalar_tensor_tensor(
                        out=zt[:], in0=zsum[:], scalar=inv_h, in1=bias_sb[:],
                        op0=Alu.mult, op1=Alu.add)
                    if act:
                        zf = wp.tile([P, C], bf, tag="zf")
                        nc.vector.scalar_tensor_tensor(
                            out=zf[:], in0=zt[:], scalar=NS_ACT, in1=zt[:],
                            op0=Alu.mult, op1=Alu.max)
                    else:
                        zf = zt
                    nc.sync.dma_start(out=z_out[t * P:(t + 1) * P, :], in_=zf[:])

            # ------------------- run -------------------
            dense_phase(wl1_sb, wr1_sb, blr1_sb, has_mbias1, xl1_tab, xr1_tab,
                        lambda lt, b0: nc.sync.dma_start(
                            out=lt[:], in_=xT[:, b0 * P:(b0 + BD) * P]))
            edge_phase(xl1_tab, xr1_tab, att1_sb, bias1_sb, z_loc, act=True)
            nc.gpsimd.collective_compute(
                "AllGather", Alu.bypass, replica_groups=[list(range(NC_))],
                ins=[z_loc.opt()], outs=[z_all.opt()])

            if dbg:
                nc.gpsimd.dma_start(out=d_xl1[:, :], in_=xl1_tab[0:2 * P, :])
                nc.gpsimd.dma_start(out=d_xr1[:, :], in_=xr1_tab[0:2 * P, :])
                nc.gpsimd.dma_start(out=d_z[:, :], in_=z_loc[:, :])
                nc.gpsimd.dma_start(out=d_zall[:, :], in_=z_all[0:2 * P, :])
            att2_sb = load_att(att2r)
            dense_phase(wl2_sb, wr2_sb, blr2_sb, has_mbias2, xl2_tab, xr2_tab,
                        lambda lt, b0: nc.sync.dma_start_transpose(
                            out=lt[:], in_=z_all[b0 * P:(b0 + BD) * P, :]))
            edge_phase(xl2_tab, xr2_tab, att2_sb, bias2_sb, z2_loc, act=False)
            nc.gpsimd.collective_compute(
                "AllGather", Alu.bypass, replica_groups=[list(range(NC_))],
                ins=[z2_loc.opt()], outs=[z2_all.opt()])

            # ------------------- P/Q tables -------------------
            for b0 in range(0, TN, BD):
                lt = wp.tile([C, BD * P], bf, tag="lhsT")
                nc.sync.dma_start_transpose(
                    out=lt[:], in_=z2_all[b0 * P:(b0 + BD) * P, :])
                pq = wp.tile([P, BD * 2 * C], bf, tag="pq")
                for i in range(BD):
                    ps = pp1.tile([P, 2 * C], f32, tag="small")
                    if has_bd1:
                        nc.tensor.matmul(out=ps[:], lhsT=ones_sb[:, :],
                                         rhs=bd1_sb[:, :], start=True, stop=False)
                        nc.tensor.matmul(out=ps[:], lhsT=lt[:, i * P:(i + 1) * P],
                                         rhs=abT_sb[:], start=False, stop=True)
                    else:
                        nc.tensor.matmul(out=ps[:], lhsT=lt[:, i * P:(i + 1) * P],
                                         rhs=abT_sb[:], start=True, stop=True)
                    nc.scalar.activation(out=pq[:, i * 2 * C:(i + 1) * 2 * C],
                                         in_=ps[:], func=Act.Copy)
                rows = slice(b0 * P, (b0 + BD) * P)
                nc.sync.dma_start(
                    out=p_tab[rows, :].rearrange("(b p) c -> p b c", p=P),
                    in_=pq[:].rearrange("p (b two c) -> p b two c", b=BD, two=2)
                        [:, :, 0, :])
                nc.sync.dma_start(
                    out=q_tab[rows, :].rearrange("(b p) c -> p b c", p=P),
                    in_=pq[:].rearrange("p (b two c) -> p b two c", b=BD, two=2)
                        [:, :, 1, :])

            # ------------------- decoder -------------------
            for j0 in range(0, DSUB, GD):
                u = gp.tile([P, GD, C], bf, tag="u")
                for j in range(GD):
                    nc.gpsimd.indirect_dma_start(
                        out=u[:, j, :], out_offset=None, in_=p_tab[:, :],
                        in_offset=bass.IndirectOffsetOnAxis(
                            ap=decS_sb[:, j0 + j:j0 + j + 1], axis=0))
                    nc.gpsimd.indirect_dma_start(
                        out=u[:, j, :], out_offset=None, in_=q_tab[:, :],
                        in_offset=bass.IndirectOffsetOnAxis(
                            ap=decD_sb[:, j0 + j:j0 + j + 1], axis=0),
                        compute_op=Alu.add)
                d = wp.tile([P, GD * C], bf, tag="d")
                uv = u[:, :, :].rearrange("p g c -> p (g c)")
                nc.vector.scalar_tensor_tensor(
                    out=d[:], in0=uv, scalar=NS_ACT, in1=uv,
                    op0=Alu.mult, op1=Alu.max)
                dw = wp.tile([P, GD * C], bf, tag="dw")
                nc.vector.tensor_tensor(out=dw[:], in0=d[:], in1=wd2_sb[:],
                                        op=Alu.mult)
                nc.vector.tensor_reduce(
                    out=res_sb[:, j0:j0 + GD],
                    in_=dw[:].rearrange("p (g c) -> p g c", g=GD),
                    axis=mybir.AxisListType.X, op=Alu.add)

            if dbg:
                nc.gpsimd.dma_start(out=d_p[:, :], in_=p_tab[0:2 * P, :])
                nc.sync.dma_start(out=d_res[:, :], in_=res_sb[:])

            # write result transposed: res_sb [P, DSUB] -> out_dec [DSUB, P]
            iotac_sb = cload(iotac, [P, 1], f32)
            ident_sb = cp.tile([P, P], f32, tag="ident")
            nc.vector.tensor_tensor(
                out=ident_sb[:],
                in0=iotac_sb[:, 0:1].to_broadcast([P, P]),
                in1=iota_sb[:, 0:P], op=Alu.is_equal)
            for b0 in range(0, DSUB, P):
                bw = min(P, DSUB - b0)
                rp = pp1.tile([P, 2 * C], f32, tag="small")
                nc.tensor.transpose(out=rp[:bw, :bw], in_=res_sb[:, b0:b0 + bw],
                                    identity=ident_sb[:])
                rsb = wp.tile([P, P], f32, tag="res_out")
                nc.scalar.activation(out=rsb[:bw, :], in_=rp[:bw, :P], func=Act.Copy)
                nc.sync.dma_start(out=out_dec[b0:b0 + bw, :], in_=rsb[:bw, :])

    if fix:
        _fix_waits(nc)
    return nc


# ---------------------------------------------------------------------------
def _rebalance_nodes(dst_counts, TN):
    """Greedy FFD: assign nodes to TN tiles of 128 nodes, equalizing edge
    counts. Returns perm (new_id -> old_id is inv; perm[old]=new)."""
    N = dst_counts.shape[0]
    order = np.argsort(-dst_counts, kind="stable")
    tile_load = np.zeros(TN, np.int64)
    tile_fill = np.zeros(TN, np.int32)
    # heap-free greedy: always put next node into least-loaded non-full tile
    import heapq
    heap = [(0, 0, t) for t in range(TN)]
    heapq.heapify(heap)
    perm = np.empty(N, np.int64)
    for node in order:
        while True:
            load, fill, t = heapq.heappop(heap)
            if tile_fill[t] < P:
                break
        perm[node] = t * P + tile_fill[t]
        tile_fill[t] += 1
        tile_load[t] += dst_counts[node]
        if tile_fill[t] < P:
            heapq.heappush(heap, (int(tile_load[t]), int(tile_fill[t]), t))
    return perm


def host_prep(inputs, n_cores=8, G=6, GD=8):
    x = np.asarray(inputs["x"], dtype=np.float32)
    N, C = x.shape
    Wl1 = np.asarray(inputs["Wl1"], np.float32)
    H = Wl1.shape[0] // C
    HC = H * C
    NPC = N // n_cores
    T = NPC // P
    TN = N // P

    ei = np.asarray(inputs["edge_index"])
    src0 = ei[0].astype(np.int64)
    dst0 = ei[1].astype(np.int64)
    E = src0.shape[0]
    loops = np.arange(N, dtype=np.int64)

    # --- node re-permutation for tile load balancing (GNN edges + loops) ---
    cnt = np.bincount(dst0, minlength=N) + 1
    perm = _rebalance_nodes(cnt, TN)          # perm[old_id] = new_id
    src = perm[src0]
    dst = perm[dst0]
    src_a = np.concatenate([src, perm[loops]])
    dst_a = np.concatenate([dst, perm[loops]])

    order = np.argsort(dst_a, kind="stable")
    src_s, dst_s = src_a[order], dst_a[order]

    tile_id = dst_s // P
    counts = np.bincount(tile_id, minlength=TN)
    S_sub = int(np.ceil(counts.max() / P))
    src_pad = np.zeros((TN, S_sub * P), np.int32)
    dstloc_pad = np.full((TN, S_sub * P), -1.0, np.float32)
    dstg_pad = np.zeros((TN, S_sub * P), np.int32)
    off = np.concatenate([[0], np.cumsum(counts)])
    for t in range(TN):
        c = counts[t]
        src_pad[t, :c] = src_s[off[t]:off[t] + c]
        dstloc_pad[t, :c] = (dst_s[off[t]:off[t] + c] - t * P).astype(np.float32)
        dstg_pad[t, :c] = dst_s[off[t]:off[t] + c]

    E_dec = E // n_cores
    assert E % n_cores == 0 and E_dec % P == 0
    DSUB = E_dec // P

    import ml_dtypes

    def bf(a):
        return np.asarray(a, np.float32).astype(ml_dtypes.bfloat16)

    def trbf(a):
        return bf(np.ascontiguousarray(np.asarray(a, np.float32).T))

    bl1 = np.asarray(inputs["bl1"], np.float32)
    br1 = np.asarray(inputs["br1"], np.float32)
    bl2 = np.asarray(inputs["bl2"], np.float32)
    br2 = np.asarray(inputs["br2"], np.float32)
    bias1 = np.asarray(inputs["bias1"], np.float32)
    bias2 = np.asarray(inputs["bias2"], np.float32)
    att1 = np.asarray(inputs["att1"], np.float32).reshape(1, HC)
    att2 = np.asarray(inputs["att2"], np.float32).reshape(1, HC)
    Wd1 = np.asarray(inputs["Wd1"], np.float32)
    bd1 = np.asarray(inputs["bd1"], np.float32)
    Wd2 = np.asarray(inputs["Wd2"], np.float32).reshape(C)

    bat = _batches(S_sub, G)
    Gmax = bat[0]
    x_perm = np.empty_like(x)
    x_perm[perm] = x                      # row new_id = x[old_id]

    b1eff = bias1 + bl1.reshape(H, C).mean(0)
    b2eff = bias2 + bl2.reshape(H, C).mean(0)
    mb1 = (bl1 + br1).reshape(1, HC)
    mb2 = (bl2 + br2).reshape(1, HC)

    shared = {
        "xT": trbf(x_perm),
        "wlT1": trbf(Wl1), "wrT1": trbf(inputs["Wr1"]),
        "wlT2": trbf(inputs["Wl2"]), "wrT2": trbf(inputs["Wr2"]),
        "att1r": bf(np.tile(att1, (P, Gmax))),
        "att2r": bf(np.tile(att2, (P, Gmax))),
        "iotar": bf(np.tile(np.arange(P, dtype=np.float32)[None, :], (P, Gmax))),
        "bias1r": np.tile(b1eff.reshape(1, C), (P, 1)).astype(np.float32),
        "bias2r": np.tile(b2eff.reshape(1, C), (P, 1)).astype(np.float32),
        "blr1": mb1, "blr2": mb2,
        "onesr": np.ones((1, P), np.float32),
        "abT": bf(np.concatenate([Wd1[:, :C].T, Wd1[:, C:].T], axis=1)),
        "bd1r": np.concatenate([bd1.reshape(1, C),
                                np.zeros((1, C), np.float32)], axis=1),
        "wd2r": bf(np.tile(Wd2.reshape(1, C), (P, GD))),
        "iotac": np.arange(P, dtype=np.float32).reshape(P, 1),
    }

    in_maps = []
    for k in range(n_cores):
        tiles = slice(k * T, (k + 1) * T)
        m = dict(shared)
        m["srcI"] = np.ascontiguousarray(
            src_pad[tiles].reshape(T * S_sub, P).T)
        m["dstloc"] = bf(np.ascontiguousarray(
            dstloc_pad[tiles].reshape(T * S_sub, P).T))
        m["dstI"] = np.ascontiguousarray(
            dstg_pad[tiles].reshape(T * S_sub, P).T)
        es = slice(k * E_dec, (k + 1) * E_dec)
        m["selfI"] = np.ascontiguousarray(
            (k * NPC + np.arange(T)[None, :] * P
             + np.arange(P)[:, None]).astype(np.int32))
        m["decS"] = np.ascontiguousarray(
            src[es].astype(np.int32).reshape(DSUB, P).T)
        m["decD"] = np.ascontiguousarray(
            dst[es].astype(np.int32).reshape(DSUB, P).T)
        in_maps.append(m)

    cfg = {
        "N": N, "C": C, "H": H, "n_cores": n_cores, "S_sub": S_sub,
        "G": G, "GD": GD, "DSUB": DSUB,
        "has_mbias1": bool(np.abs(mb1).max() > 0),
        "has_mbias2": bool(np.abs(mb2).max() > 0),
        "has_bd1": bool(np.abs(bd1).max() > 0),
        "bd2": float(np.asarray(inputs["bd2"]).reshape(-1)[0]),
    }
    return in_maps, cfg


def postprocess(results, cfg):
    n_cores = cfg["n_cores"]
    out = np.concatenate(
        [results[k]["out_dec"].reshape(-1) for k in range(n_cores)])
    return (out + cfg["bd2"]).astype(np.float32)


def kernel(**inputs):
    from concourse.bass_utils import run_bass_kernel_spmd

    n_cores = 8
    in_maps, cfg = host_prep(inputs, n_cores=n_cores)
    nc = build_program(cfg)
    res = run_bass_kernel_spmd(nc, in_maps, list(range(n_cores)))
    return postprocess(res.results, cfg)
